# revision 12
# baseline (speedup 1.0000x reference)
"""Trainium2 Bass kernel for nn_DiffusionMemory: per-frame spatial attention
(8 heads, head_dim 32, 1024 positions) + kernel-3 temporal conv, C=256, T=16.

Sharding: T axis across 8 cores (2 frames/core), halo of 1 frame for the conv.

Per-core dataflow (all fp32):
  - Q,K channel-major [256, 1024] via PE (weights pre-transposed host-side,
    pos_emb folded into per-frame biases, attn scale folded into wq).
  - V position-major [1024, 256] via PE, written into an "augmented" SBUF
    layout with a ones-column per head (ones-trick computes the softmax
    denominator inside the AV matmul).
  - Scores computed key-major S^T[q', p] with 2-head row-packing (K=32).
  - exp on ACT directly PSUM->SBUF (only ACT user; it is the bottleneck).
  - AV with 2-head col-packing (M=64 strips); row 32/96 of the PSUM output
    are the per-head softmax denominators.
  - normalize via vector.reciprocal + gpsimd.partition_broadcast + DVE mul.
  - final = temporal conv + bias + o-proj accumulated in one PSUM tile,
    DMA'd straight to DRAM.
"""

import sys

if "/opt/trn_rl_repo" not in sys.path:
    sys.path.insert(0, "/opt/trn_rl_repo")

import numpy as np

C = 256
NH = 8
HD = 32
T = 16
P = 1024  # positions per frame (32*32)
N_CORES = 8
FPC = 2  # frames per core

_CACHE = {}


def _patch_tile_drain():
    """This walrus build allows only 1 sync-wait per Drain/NOP; split the
    TileContext final drain's waits across nofuse NOPs."""
    import concourse.mybir as mybir
    import concourse.tile as tile

    if getattr(tile.TileContext, "_drain_patched", False):
        return

    def _drain_and_barrier(self, tick_clock, wait_clock):
        from concourse.tile import ScopedClock

        nc = self.nc
        drain_inst = nc.sync.drain()
        wait_clock.add_sem_waits(
            drain_inst.ins, ScopedClock({None: tick_clock.global_clock})
        )
        si = drain_inst.ins.sync_info
        waits = list(si.on_wait or [])
        if len(waits) > 1:
            si.on_wait = waits[:1]
            for w in waits[1:]:
                nop = nc.sync.nop(nofuse=True)
                nsi = nop.ins.sync_info
                if nsi is None:
                    nsi = mybir.SyncInfo(on_wait=[], on_update=[])
                    nop.ins.sync_info = nsi
                nsi.on_wait = [w]
        nc.all_engine_barrier()
        popped = nc._tile_sem_poison_stack.pop()
        assert popped is self._sem_poison
        nc.clear_and_free_semaphores(list(self.sems.allocated().values()))
        nc.all_engine_barrier()

    tile.TileContext._drain_and_barrier = _drain_and_barrier
    tile.TileContext._drain_patched = True


def _patch_compile_split_waits():
    """This walrus build allows only one sync-wait command per instruction.
    Post-process the BIR JSON before compile: hoist extra waits onto NoOps
    inserted just before the instruction on the same engine."""
    import json

    import concourse.bass_utils as bass_utils
    import concourse.bass2jax as bass2jax

    if getattr(bass_utils, "_split_waits_patched", False):
        return
    orig = bass_utils.compile_bir_kernel

    def _split(bir_bytes):
        bir = json.loads(bir_bytes)
        ctr = [0]

        def fix_block(b):
            insts = b.get("instructions")
            if insts:
                new = []
                for inst in insts:
                    si = inst.get("sync_info") or {}
                    w = si.get("on_wait") or []
                    if len(w) > 1:
                        for extra in w[:-1]:
                            ctr[0] += 1
                            new.append(
                                {
                                    "debug": inst.get("debug", 0),
                                    "engine": inst["engine"],
                                    "ins": [],
                                    "name": f"I-wsplit-{ctr[0]}",
                                    "opcode": "NoOp",
                                    "outs": [],
                                    "sync_info": {
                                        "on_update": [],
                                        "on_wait": [extra],
                                    },
                                }
                            )
                        si["on_wait"] = [w[-1]]
                    new.append(inst)
                b["instructions"] = new
            for sb in b.get("blocks") or []:
                fix_block(sb)

        for f in bir["functions"]:
            for blk in f["blocks"]:
                fix_block(blk)
        return json.dumps(bir).encode()

    def patched(bir_json, tmpdir, neff_name="file.neff"):
        return orig(_split(bir_json), tmpdir, neff_name)

    bass_utils.compile_bir_kernel = patched
    bass2jax.compile_bir_kernel = patched
    bass_utils._split_waits_patched = True


def _build_nc():
    import concourse.bass as bass
    import concourse.mybir as mybir
    import concourse.tile as tile

    _patch_tile_drain()
    _patch_compile_split_waits()
    f32 = mybir.dt.float32
    Exp = mybir.ActivationFunctionType.Exp

    nc = bass.Bass("TRN2", target_bir_lowering=False, debug=False)

    xs_d = nc.dram_tensor("xs", [2, 128, 4, P], f32, kind="ExternalInput")
    wq_d = nc.dram_tensor("wq", [2, 128, C], f32, kind="ExternalInput")
    wk_d = nc.dram_tensor("wk", [2, 128, C], f32, kind="ExternalInput")
    wv_d = nc.dram_tensor("wv", [2, 128, C], f32, kind="ExternalInput")
    wo_d = nc.dram_tensor("wo", [2, 128, C], f32, kind="ExternalInput")
    tc_d = nc.dram_tensor("tc", [3, 2, 128, C], f32, kind="ExternalInput")
    qb_d = nc.dram_tensor("qb", [FPC, 2, 128], f32, kind="ExternalInput")
    kb_d = nc.dram_tensor("kb", [FPC, 2, 128], f32, kind="ExternalInput")
    vb_d = nc.dram_tensor("vb", [FPC, C], f32, kind="ExternalInput")
    ob_d = nc.dram_tensor("ob", [1, C], f32, kind="ExternalInput")
    out_d = nc.dram_tensor("out", [2, 128, FPC, P], f32, kind="ExternalOutput")

    with tile.TileContext(nc) as tc:
        import contextlib

        ctx = contextlib.ExitStack()
        with ctx:
            consts = ctx.enter_context(tc.tile_pool(name="consts", bufs=1))
            xsp = ctx.enter_context(tc.tile_pool(name="xsp", bufs=1))
            qsp = ctx.enter_context(tc.tile_pool(name="qsp", bufs=2))
            ksp = ctx.enter_context(tc.tile_pool(name="ksp", bufs=2))
            vap = ctx.enter_context(tc.tile_pool(name="vap", bufs=10))
            ep = ctx.enter_context(tc.tile_pool(name="ep", bufs=8))
            ynp = ctx.enter_context(tc.tile_pool(name="ynp", bufs=3))
            rcp = ctx.enter_context(tc.tile_pool(name="rcp", bufs=6))
            fop = ctx.enter_context(tc.tile_pool(name="fop", bufs=3))
            ps_s = ctx.enter_context(tc.tile_pool(name="ps_s", bufs=2, space="PSUM"))
            ps_y = ctx.enter_context(tc.tile_pool(name="ps_y", bufs=2, space="PSUM"))
            ps_m1 = ctx.enter_context(tc.tile_pool(name="ps_m1", bufs=1, space="PSUM"))
            ps_m2 = ctx.enter_context(tc.tile_pool(name="ps_m2", bufs=1, space="PSUM"))

            # ---- constants / weights ----
            xs_sb = []
            for kt in range(2):
                t_ = xsp.tile([128, 4, P], f32, tag=f"xs{kt}")
                nc.sync.dma_start(out=t_, in_=xs_d.ap()[kt])
                xs_sb.append(t_)

            def load_w(dram):
                tiles = []
                for kt in range(2):
                    t_ = consts.tile([128, C], f32, tag=f"w{dram.name}{kt}")
                    nc.sync.dma_start(out=t_, in_=dram.ap()[kt])
                    tiles.append(t_)
                return tiles

            wq_sb = load_w(wq_d)
            wk_sb = load_w(wk_d)
            wv_sb = load_w(wv_d)
            wo_sb = load_w(wo_d)
            tc_sb = []
            for kt in range(2):
                t_ = consts.tile([128, 3, C], f32, tag=f"tc{kt}")
                for tap in range(3):
                    nc.sync.dma_start(out=t_[:, tap, :], in_=tc_d.ap()[tap, kt])
                tc_sb.append(t_)

            qb_sb = consts.tile([128, FPC, 2], f32, tag="qb")
            kb_sb = consts.tile([128, FPC, 2], f32, tag="kb")
            for f in range(FPC):
                for ct in range(2):
                    nc.sync.dma_start(
                        out=qb_sb[:, f, ct : ct + 1], in_=qb_d.ap()[f, ct][:, None]
                    )
                    nc.sync.dma_start(
                        out=kb_sb[:, f, ct : ct + 1], in_=kb_d.ap()[f, ct][:, None]
                    )
            vb_sb = consts.tile([1, FPC, C], f32, tag="vb")
            nc.sync.dma_start(out=vb_sb, in_=vb_d.ap()[None])
            ob_sb = consts.tile([1, C], f32, tag="ob")
            nc.sync.dma_start(out=ob_sb, in_=ob_d.ap())
            ones_sb = consts.tile([1, 512], f32, tag="ones")
            nc.vector.memset(ones_sb, 1.0)

            # ---- per-frame program ----
            for f in range(FPC):
                # Q/K projections, channel-major [c_out, p]
                q_tiles, k_tiles = [], []
                for which, w_sb, b_sb, outl in (
                    ("q", wq_sb, qb_sb, q_tiles),
                    ("k", wk_sb, kb_sb, k_tiles),
                ):
                    for ct in range(2):
                        dst = (qsp if which == "q" else ksp).tile(
                            [128, P], f32, tag=f"{which}s"
                        )
                        for ph in range(2):
                            ps = ps_m1.tile([128, 512], f32, tag="m1")
                            for kt in range(2):
                                nc.tensor.matmul(
                                    ps,
                                    w_sb[kt][:, ct * 128 : (ct + 1) * 128],
                                    xs_sb[kt][:, f + 1, ph * 512 : (ph + 1) * 512],
                                    start=(kt == 0),
                                    stop=(kt == 1),
                                )
                            nc.vector.tensor_scalar_add(
                                dst[:, ph * 512 : (ph + 1) * 512],
                                ps,
                                b_sb[:, f, ct : ct + 1],
                            )
                        outl.append(dst)

                # V projection, position-major, into augmented layout.
                # vaug[qt] free layout, per pair-block j (128 cols):
                #   [0:32]=v_head(2j), [32:64]=ones, [64:96]=v_head(2j+1),
                #   [96:128]=ones. The 32 ones-columns replicate the softmax
                #   denominator across 32 PSUM partitions, so reciprocal()
                #   directly yields the [32, 512] normalizer (no broadcast).
                vaug = []
                for qt in range(8):
                    va = vap.tile([128, 4, 128], f32, tag="vaug")
                    nc.gpsimd.memset(va[:, :, 32:64], 1.0)
                    nc.gpsimd.memset(va[:, :, 96:128], 1.0)
                    ps = ps_m1.tile([128, C], f32, tag="m1")
                    for kt in range(2):
                        nc.tensor.matmul(
                            ps,
                            xs_sb[kt][:, f + 1, qt * 128 : (qt + 1) * 128],
                            wv_sb[kt],
                            start=(kt == 0),
                            stop=False,
                        )
                    # + pos-emb-folded bias (varies along free axis): ones-row MM
                    nc.tensor.matmul(
                        ps,
                        ones_sb[:, 0:128],
                        vb_sb[:, f, :],
                        start=False,
                        stop=True,
                    )
                    # evac: even heads -> col offset 0, odd heads -> offset 64
                    ve = ps.rearrange("p (h d) -> p h d", d=HD)
                    nc.vector.tensor_copy(va[:, :, 0:32], ve[:, 0::2, :])
                    nc.vector.tensor_copy(va[:, :, 64:96], ve[:, 1::2, :])
                    vaug.append(va)

                yn_tiles = []
                for _ct in range(2):
                    yn_t = ynp.tile([128, P], f32, tag="yn")
                    yn_tiles.append(yn_t)

                for j in range(4):  # head pairs (2j, 2j+1)
                    ktile = j // 2
                    A = 64 * (j % 2)
                    B = A + 32
                    e_tiles = []
                    for qt in range(8):
                        sa = ps_s.tile([128, P], f32, tag="s")
                        sb_ = ps_s.tile([128, P], f32, tag="s")
                        for ph in range(2):
                            sl = slice(ph * 512, (ph + 1) * 512)
                            nc.tensor.matmul(
                                sa[:, sl],
                                k_tiles[ktile][A:B, qt * 128 : (qt + 1) * 128],
                                q_tiles[ktile][A:B, sl],
                                start=True,
                                stop=True,
                                tile_position=(A, 0),
                            )
                            nc.tensor.matmul(
                                sb_[:, sl],
                                k_tiles[ktile][B : B + 32, qt * 128 : (qt + 1) * 128],
                                q_tiles[ktile][B : B + 32, sl],
                                start=True,
                                stop=True,
                                tile_position=(B, 0),
                            )
                        et = ep.tile([128, 2 * P], f32, tag="e")
                        nc.scalar.activation(et[:, 0:P], sa, Exp)
                        nc.scalar.activation(et[:, P : 2 * P], sb_, Exp)
                        e_tiles.append(et)

                    for ph in range(2):
                        sl = slice(ph * 512, (ph + 1) * 512)
                        yps = ps_y.tile([128, 512], f32, tag="y")
                        for qt in range(8):
                            nc.tensor.matmul(
                                yps[0:64, :],
                                vaug[qt][:, j, 0:64],
                                e_tiles[qt][:, sl],
                                start=(qt == 0),
                                stop=(qt == 7),
                                tile_position=(0, 0),
                                skip_group_check=True,
                            )
                            nc.tensor.matmul(
                                yps[64:128, :],
                                vaug[qt][:, j, 64:128],
                                e_tiles[qt][:, P + ph * 512 : P + (ph + 1) * 512],
                                start=(qt == 0),
                                stop=(qt == 7),
                                tile_position=(0, 64),
                                skip_group_check=True,
                            )
                        for hh in range(2):  # head 2j+hh
                            g = 2 * j + hh
                            base = 64 * hh
                            rb = rcp.tile([32, 512], f32, tag="rc")
                            nc.vector.reciprocal(rb, yps[base + 32 : base + 64, :])
                            nc.vector.tensor_mul(
                                yn_tiles[g // 4][
                                    32 * (g % 4) : 32 * (g % 4) + 32, sl
                                ],
                                yps[base : base + 32, :],
                                rb,
                            )

                # final: conv + bias + o-proj accumulated in PSUM, DMA out
                for ct in range(2):
                    for ph in range(2):
                        sl = slice(ph * 512, (ph + 1) * 512)
                        fin = ps_m2.tile([128, 512], f32, tag="m2")
                        first = True
                        for tap in range(3):
                            for kt in range(2):
                                nc.tensor.matmul(
                                    fin,
                                    tc_sb[kt][:, tap, ct * 128 : (ct + 1) * 128],
                                    xs_sb[kt][:, f + tap, sl],
                                    start=first,
                                    stop=False,
                                )
                                first = False
                        nc.tensor.matmul(
                            fin,
                            ob_sb[:, ct * 128 : (ct + 1) * 128],
                            ones_sb,
                            start=False,
                            stop=False,
                        )
                        for kt in range(2):
                            nc.tensor.matmul(
                                fin,
                                wo_sb[kt][:, ct * 128 : (ct + 1) * 128],
                                yn_tiles[kt][:, sl],
                                start=False,
                                stop=(kt == 1),
                            )
                        fo = fop.tile([128, 512], f32, tag="fo")
                        nc.vector.tensor_copy(fo, fin)
                        nc.sync.dma_start(out=out_d.ap()[ct, :, f, sl], in_=fo)
    return nc


def _prep_inputs(x, tc_w, tc_b, q_w, q_b, k_w, k_b, v_w, v_b, o_w, o_b, pos_emb):
    scale = np.float32(HD**-0.5)
    x3 = np.asarray(x, np.float32).reshape(C, T, P)
    xpad = np.zeros((C, T + 2, P), np.float32)
    xpad[:, 1 : T + 1] = x3
    pe = np.asarray(pos_emb, np.float32).reshape(C, T)

    qbias = ((q_w @ pe) + q_b[:, None]) * scale  # [C, T]
    kbias = (k_w @ pe) + k_b[:, None]
    vbias = (v_w @ pe) + v_b[:, None]
    wq = np.ascontiguousarray((q_w.T * scale).reshape(2, 128, C), np.float32)
    wk = np.ascontiguousarray(k_w.T.reshape(2, 128, C), np.float32)
    wv = np.ascontiguousarray(v_w.T.reshape(2, 128, C), np.float32)
    wo = np.ascontiguousarray(o_w.T.reshape(2, 128, C), np.float32)
    tcT = np.ascontiguousarray(
        tc_w.transpose(2, 1, 0).reshape(3, 2, 128, C), np.float32
    )
    ob = np.ascontiguousarray((tc_b + o_b).reshape(1, C), np.float32)

    in_maps = []
    for i in range(N_CORES):
        fr = [2 * i, 2 * i + 1]
        in_maps.append(
            {
                "xs": np.ascontiguousarray(
                    xpad[:, 2 * i : 2 * i + 4].reshape(2, 128, 4, P)
                ),
                "wq": wq,
                "wk": wk,
                "wv": wv,
                "wo": wo,
                "tc": tcT,
                "qb": np.ascontiguousarray(
                    qbias[:, fr].T.reshape(FPC, 2, 128), np.float32
                ),
                "kb": np.ascontiguousarray(
                    kbias[:, fr].T.reshape(FPC, 2, 128), np.float32
                ),
                "vb": np.ascontiguousarray(vbias[:, fr].T, np.float32),
                "ob": ob,
            }
        )
    return in_maps


def _build_sharded_callable(nc):
    """Reusable jitted SPMD callable mirroring bass2jax.run_bass_via_pjrt's
    multi-core path, for benchmarking with device-resident inputs."""
    import jax
    import numpy as _np
    from jax.sharding import Mesh, PartitionSpec
    from jax.experimental.shard_map import shard_map
    import concourse.mybir as mybir
    from concourse.bass2jax import (
        _bass_exec_p,
        install_neuronx_cc_hook,
        partition_id_tensor,
    )

    install_neuronx_cc_hook()
    partition_name = nc.partition_id_tensor.name if nc.partition_id_tensor else None
    in_names, out_names, out_avals, zero_outs = [], [], [], []
    for alloc in nc.m.functions[0].allocations:
        if not isinstance(alloc, mybir.MemoryLocationSet):
            continue
        name = alloc.memorylocations[0].name
        if alloc.kind == "ExternalInput":
            if name != partition_name:
                in_names.append(name)
        elif alloc.kind == "ExternalOutput":
            shape = tuple(alloc.tensor_shape)
            dtype = mybir.dt.np(alloc.dtype)
            out_names.append(name)
            out_avals.append(jax.core.ShapedArray(shape, dtype))
            zero_outs.append(_np.zeros(shape, dtype))
    n_params = len(in_names)
    all_names = in_names + out_names
    if partition_name is not None:
        all_names = all_names + [partition_name]

    def _body(*args):
        operands = list(args)
        if partition_name is not None:
            operands.append(partition_id_tensor())
        outs = _bass_exec_p.bind(
            *operands,
            out_avals=tuple(out_avals),
            in_names=tuple(all_names),
            out_names=tuple(out_names),
            lowering_input_output_aliases=(),
            sim_require_finite=True,
            sim_require_nnan=True,
            nc=nc,
        )
        return tuple(outs)

    devices = jax.devices()[:N_CORES]
    mesh = Mesh(_np.asarray(devices), ("core",))
    nall = n_params + len(out_names)
    sharded = jax.jit(
        shard_map(
            _body,
            mesh=mesh,
            in_specs=(PartitionSpec("core"),) * nall,
            out_specs=(PartitionSpec("core"),) * len(out_names),
            check_rep=False,
        ),
        donate_argnums=tuple(range(n_params, nall)),
        keep_unused=True,
    )
    return sharded, mesh, in_names, out_names, zero_outs


def bench(inputs, iters=30):
    """Time repeated kernel executions with device-resident inputs.
    Returns the best per-iteration wall time in ns (upper bound on HW time)."""
    import time

    import jax
    import numpy as _np
    from jax.sharding import NamedSharding, PartitionSpec

    if "nc" not in _CACHE:
        _CACHE["nc"] = _build_nc()
    nc = _CACHE["nc"]
    in_maps = _prep_inputs(**inputs)
    sharded, mesh, in_names, out_names, zero_outs = _build_sharded_callable(nc)
    spec = NamedSharding(mesh, PartitionSpec("core"))
    concat_in = [
        _np.concatenate([in_maps[c][n] for c in range(N_CORES)], axis=0)
        for n in in_names
    ]
    dev_in = [jax.device_put(a, spec) for a in concat_in]
    jax.block_until_ready(dev_in)

    times = []
    out = None
    for it in range(iters):
        zo = [
            jax.device_put(
                _np.zeros((N_CORES * z.shape[0], *z.shape[1:]), z.dtype), spec
            )
            for z in zero_outs
        ]
        jax.block_until_ready(zo)
        t0 = time.perf_counter()
        out = sharded(*dev_in, *zo)
        jax.block_until_ready(out)
        times.append(time.perf_counter() - t0)
    times_ns = sorted(t * 1e9 for t in times)
    print(
        f"bench wall-times ns: min={times_ns[0]:.0f} p50={times_ns[len(times_ns)//2]:.0f} "
        f"max={times_ns[-1]:.0f}"
    )
    return times_ns[0]


def kernel(x, tc_w, tc_b, q_w, q_b, k_w, k_b, v_w, v_b, o_w, o_b, pos_emb):
    from concourse.bass_utils import run_bass_kernel_spmd

    if "nc" not in _CACHE:
        _CACHE["nc"] = _build_nc()
    nc = _CACHE["nc"]

    in_maps = _prep_inputs(
        x, tc_w, tc_b, q_w, q_b, k_w, k_b, v_w, v_b, o_w, o_b, pos_emb
    )
    res = run_bass_kernel_spmd(nc, in_maps, core_ids=list(range(N_CORES)))
    outs = []
    for i in range(N_CORES):
        outs.append(res.results[i]["out"].reshape(C, FPC, P))
    full = np.concatenate(outs, axis=1)  # [C, 16, P]
    return np.ascontiguousarray(full.reshape(1, C, T, 32, 32), np.float32)
